# revision 1
# baseline (speedup 1.0000x reference)
"""MoE expert-parallel kernel for Trainium2 (8 NeuronCores, 1 expert/core).

Reference computation per expert e:
    h   = relu(x_e @ W1_e)               [N, DFF]
    agg[d] += h[src[k]] for dst[k]==d    (segment-sum over NE edges)
    out = agg @ W2_e                     [N, D]

Key transformations:
  1. segment_sum is linear:  (S @ h) @ W2 == S @ (h @ W2),
     where S[d, s] = #edges s->d.  Applying W2 *before* the aggregation
     halves the cost of the aggregation matmul (D < DFF).
  2. S is built on the host from edge_index (dense count matrix) so the
     gather/scatter becomes a dense matmul on the tensor engine.
  3. All matmuls run in float32r (full-rate fp32 on the PE at free-dim
     >= 256, ~1e-4 relative error vs fp32).

Device pipeline per core (expert):
    phase A: hT[f, n] = relu( W1[d, f].T @ xT[d, n] )       (K = D)
    phase B: m[n, d]  = hT[f, n].T @ W2[f, d]               (K = DFF)
    phase C: out[n', d] = ST[s, n'].T @ m[s, d]             (K = N)
xT, a per-f-tile W1 layout, and a per-n-tile ST layout are prepared on
the host.  hT round-trips through internal DRAM; m is written by phase B
directly into 30 SBUF-resident chunks (the last 2 bounce through DRAM —
no SBUF room for them while W2 is resident).  W1/W2 chunks are staged
just-in-time so matmuls start ~8us in instead of waiting for weights.
"""

import os
from contextlib import ExitStack

import numpy as np

import concourse.bass as bass
import concourse.mybir as mybir
import concourse.tile as tile
from concourse import bacc
from concourse.bass_utils import run_bass_kernel_spmd

E, N, D, DFF = 8, 4096, 1024, 2048
P = 128
NT = N // P     # 32  n tiles
DC = D // P     # 8   d chunks (K for phase A)
FT = DFF // P   # 16  f tiles
DS = D // 512   # 2   d slices of 512
NS = N // 512   # 8   n slices of 512
M_DIRECT = 30   # m chunks written straight to SBUF during phase B

F32 = mybir.dt.float32
F32R = mybir.dt.float32r
RELU = mybir.ActivationFunctionType.Relu

_cache = {}


def _emit_rep(nc, tc, r, xT, W1H, W2, ST, out, hT, mD31):
    """Emit one full pipeline (phases A/B/C) with rep-unique pool names."""
    W2r = W2.rearrange("(fc p) d -> p fc d", p=P)
    with tc.tile_pool(name=f"w2h0p{r}", bufs=1) as w2h0p:
        # d-half 0 of W2: staged while phase A runs
        w2h0 = w2h0p.tile([P, FT, 512], F32R, name=f"w2h0_{r}")

        # ---------- phase A: hT = relu(W1.T @ xT) ----------
        with tc.tile_pool(name=f"w1p{r}", bufs=1) as w1p, \
             tc.tile_pool(name=f"xp{r}", bufs=2) as xp, \
             tc.tile_pool(name=f"hp{r}", bufs=3) as hp, \
             tc.tile_pool(name=f"psA{r}", bufs=4, space="PSUM") as psA:
            w1sb = w1p.tile([P, FT, DC, P], F32R, name=f"w1sb_{r}")
            xTr = xT.rearrange("(dc p) n -> p dc n", p=P)
            xsbs = [
                xp.tile([P, DC, 512], F32R, tag="xsb", name=f"xsb{ns}_{r}")
                for ns in range(NS)
            ]
            nc.sync.dma_start(out=xsbs[0][:], in_=xTr[:, :, 0:512])
            nc.sync.dma_start(out=w1sb[:, 0], in_=W1H[0])
            for ns in range(NS):
                xsb = xsbs[ns]
                for ft in range(FT):
                    pt = psA.tile([P, 512], F32, name=f"ptA_{r}")
                    for dc in range(DC):
                        nc.tensor.matmul(
                            out=pt[:],
                            lhsT=w1sb[:, ft, dc, :],
                            rhs=xsb[:, dc, :],
                            start=(dc == 0),
                            stop=(dc == DC - 1),
                        )
                    hsb = hp.tile([P, 512], F32, name=f"hsb_{r}")
                    nc.scalar.activation(out=hsb[:], in_=pt[:], func=RELU)
                    nc.sync.dma_start(
                        out=hT[ft * P : (ft + 1) * P, ns * 512 : (ns + 1) * 512],
                        in_=hsb[:].bitcast(F32R),
                    )
                    if ns == 0 and ft + 1 < FT:
                        # JIT-stage the next W1 chunk behind this group
                        nc.sync.dma_start(out=w1sb[:, ft + 1], in_=W1H[ft + 1])
                    if ft == 0 and ns + 1 < NS:
                        # prefetch next x slice early so it isn't queued
                        # behind this iteration's hT writes
                        nc.sync.dma_start(
                            out=xsbs[ns + 1][:],
                            in_=xTr[:, :, (ns + 1) * 512 : (ns + 2) * 512],
                        )
                # stage W2 d-half 0 during phase A (2 chunks per n slice)
                for fc in (2 * ns, 2 * ns + 1):
                    nc.sync.dma_start(out=w2h0[:, fc], in_=W2r[:, fc, 0:512])

        # 30 resident m chunks, filled by phase B, read by phase C
        with tc.tile_pool(name=f"mp{r}", bufs=1) as mp:
            msb = [None] * NT

            # ---------- phase B: m = hT.T @ W2 ----------
            with tc.tile_pool(name=f"w2h1p{r}", bufs=1) as w2h1p, \
                 tc.tile_pool(name=f"htp{r}", bufs=2) as htp, \
                 tc.tile_pool(name=f"mbp{r}", bufs=1) as mbp, \
                 tc.tile_pool(name=f"psB{r}", bufs=4, space="PSUM") as psB:
                # d-half 1 of W2: JIT-loaded at B start (ds=0 groups only
                # need w2h0, so the PE is busy while this streams in)
                w2h1 = w2h1p.tile([P, FT, 512], F32R, name=f"w2h1_{r}")
                w2h = [w2h0, w2h1]
                hTr = hT.rearrange("(fc p) n -> p fc n", p=P)
                htsbs = [
                    htp.tile([P, FT, P], F32R, tag="htsb", name=f"htsb{nt}_{r}")
                    for nt in range(NT)
                ]
                nc.sync.dma_start(out=htsbs[0][:], in_=hTr[:, :, 0:P])
                for fc in range(FT):
                    nc.sync.dma_start(out=w2h1[:, fc], in_=W2r[:, fc, 512:1024])
                for nt in range(NT):
                    htsb = htsbs[nt]
                    if nt + 1 < NT:
                        nc.sync.dma_start(
                            out=htsbs[nt + 1][:],
                            in_=hTr[:, :, (nt + 1) * P : (nt + 2) * P],
                        )
                    if nt < M_DIRECT:
                        msb[nt] = mp.tile(
                            [P, D], F32R, tag=f"m{nt}", name=f"msb{nt}_{r}"
                        )
                    else:
                        mb = mbp.tile([P, D], F32, tag="mb", name=f"mb_{r}")
                    for ds in range(DS):
                        pt = psB.tile([P, 512], F32, name=f"ptB_{r}")
                        for fc in range(FT):
                            nc.tensor.matmul(
                                out=pt[:],
                                lhsT=htsb[:, fc, :],
                                rhs=w2h[ds][:, fc, :],
                                start=(fc == 0),
                                stop=(fc == FT - 1),
                            )
                        if nt < M_DIRECT:
                            # write m straight into its resident chunk
                            nc.vector.tensor_copy(
                                out=msb[nt][:, ds * 512 : (ds + 1) * 512],
                                in_=pt[:],
                            )
                        else:
                            nc.vector.tensor_copy(
                                out=mb[:, ds * 512 : (ds + 1) * 512], in_=pt[:]
                            )
                    if nt >= M_DIRECT:
                        nc.sync.dma_start(
                            out=mD31[
                                (nt - M_DIRECT) * P : (nt - M_DIRECT + 1) * P, :
                            ],
                            in_=mb[:].bitcast(F32R),
                        )

            # ---------- phase C: out = ST.T @ m ----------
            with tc.tile_pool(name=f"m3p{r}", bufs=1) as m3p, \
                 tc.tile_pool(name=f"stp{r}", bufs=2) as stp, \
                 tc.tile_pool(name=f"op{r}", bufs=3) as op, \
                 tc.tile_pool(name=f"psC{r}", bufs=4, space="PSUM") as psC:
                for nt in range(M_DIRECT, NT):
                    msb[nt] = m3p.tile(
                        [P, D], F32R, tag=f"m{nt}", name=f"msb{nt}_{r}"
                    )
                    nc.sync.dma_start(
                        out=msb[nt][:],
                        in_=mD31[(nt - M_DIRECT) * P : (nt - M_DIRECT + 1) * P, :],
                    )
                stsbs = [
                    stp.tile([P, NT, P], F32R, tag="stsb", name=f"stsb{nt}_{r}")
                    for nt in range(NT)
                ]
                nc.sync.dma_start(out=stsbs[0][:], in_=ST[0])
                for nt in range(NT):
                    stsb = stsbs[nt]
                    if nt + 1 < NT:
                        nc.sync.dma_start(out=stsbs[nt + 1][:], in_=ST[nt + 1])
                    for ds in range(DS):
                        pt = psC.tile([P, 512], F32, name=f"ptC_{r}")
                        for sc in range(NT):
                            nc.tensor.matmul(
                                out=pt[:],
                                lhsT=stsb[:, sc, :],
                                rhs=msb[sc][:, ds * 512 : (ds + 1) * 512],
                                start=(sc == 0),
                                stop=(sc == NT - 1),
                            )
                        osb = op.tile([P, 512], F32, name=f"osb_{r}")
                        nc.vector.tensor_copy(out=osb[:], in_=pt[:])
                        nc.sync.dma_start(
                            out=out[
                                nt * P : (nt + 1) * P, ds * 512 : (ds + 1) * 512
                            ],
                            in_=osb[:],
                        )


def _build(reps=1):
    nc = bacc.Bacc()

    xT = nc.dram_tensor("xT", [D, N], F32R, kind="ExternalInput")
    # W1H[ft, p, dc, f'] = W1[dc*128 + p, ft*128 + f']  (host-tiled: one
    # 512KB chunk per f-tile, so the first matmul group waits on ~2.5MB
    # of DMA, not 8MB)
    W1H = nc.dram_tensor("W1H", [FT, P, DC, P], F32R, kind="ExternalInput")
    W2 = nc.dram_tensor("W2", [DFF, D], F32R, kind="ExternalInput")
    # ST[nt, p, sc, n'] = S_T[sc*128 + p, nt*128 + n']  (host-tiled so each
    # phase-C load is one contiguous 16KB line per partition)
    ST = nc.dram_tensor("ST", [NT, P, NT, P], F32R, kind="ExternalInput")
    out = nc.dram_tensor("out", [N, D], F32, kind="ExternalOutput")

    with tile.TileContext(nc) as tc:
        with tc.tile_pool(name="dram", bufs=1, space="DRAM") as dram:
            hT = dram.tile([DFF, N], F32R)
            mD31 = dram.tile([2 * P, D], F32R)
            for r in range(reps):
                _emit_rep(nc, tc, r, xT, W1H, W2, ST, out, hT, mD31)

    nc.compile()
    return nc


def kernel(x, W1, W2, edge_index):
    x = np.asarray(x, dtype=np.float32)
    W1 = np.asarray(W1, dtype=np.float32)
    W2 = np.asarray(W2, dtype=np.float32)
    edge_index = np.asarray(edge_index)

    # S_T[s, d] = #edges with src==s and dst==d  (so out = S_T.T @ m)
    src = edge_index[0].astype(np.int64)
    dst = edge_index[1].astype(np.int64)
    counts = np.bincount(src * N + dst, minlength=N * N)
    S_T = counts.reshape(N, N).astype(np.float32)
    # host tiling for contiguous phase-C DMA: [nt, p, sc, n']
    STH = np.ascontiguousarray(S_T.reshape(NT, P, NT, P).transpose(2, 1, 0, 3))

    if "nc" not in _cache:
        _cache["nc"] = _build()
    nc = _cache["nc"]

    in_maps = []
    for e in range(E):
        # W1H[ft, p, dc, f'] = W1[e, dc*128+p, ft*128+f']
        W1H = np.ascontiguousarray(
            W1[e].reshape(DC, P, FT, P).transpose(2, 1, 0, 3)
        )
        in_maps.append(
            {
                "xT": np.ascontiguousarray(x[e].T),
                "W1H": W1H,
                "W2": np.ascontiguousarray(W2[e]),
                "ST": STH,
            }
        )

    trace = bool(int(os.environ.get("PROBLEM_TRACE", "0")))
    res = run_bass_kernel_spmd(nc, in_maps, core_ids=list(range(E)), trace=trace)
    _cache["last_results"] = res
    return np.stack([res.results[e]["out"] for e in range(E)]).astype(np.float32)



# revision 4
# speedup vs baseline: 1.1450x; 1.1450x over previous
"""MoE expert-parallel kernel for Trainium2 (8 NeuronCores, 1 expert/core).

Reference computation per expert e:
    h   = relu(x_e @ W1_e)               [N, DFF]
    agg[d] += h[src[k]] for dst[k]==d    (segment-sum over NE edges)
    out = agg @ W2_e                     [N, D]

Key transformations:
  1. segment_sum is linear:  (S @ h) @ W2 == S @ (h @ W2),
     where S[d, s] = #edges s->d.  Applying W2 *before* the aggregation
     halves the cost of the aggregation matmul (D < DFF).
  2. S is built on the host from edge_index (dense count matrix) so the
     gather/scatter becomes a dense matmul on the tensor engine.
  3. All matmul inputs are bf16 (fp32 PSUM accumulation).  bf16 runs at
     the same 1 row/cycle PE rate as fp32r but halves SBUF footprint and
     HBM traffic, and keeps Fast Weight Load eligible so LDWEIGHTS hides
     behind the previous matmul.  End-to-end error ~3e-3 vs the fp32
     reference (gate is 2e-2).

Device pipeline per core (expert), single fused pass:
    for each 512-token slice:  hT = relu(W1.T @ xT_slice)   (K = D)
                               m_slice = hT.T @ W2          (K = DFF)
    out = ST.T @ m                                          (K = N)
W1 (4.2 MB), W2 (4.2 MB) and all of m (8.4 MB) stay SBUF-resident, so h
never round-trips through DRAM and the PE runs back-to-back from the
first matmul to the last.  Weights are staged just-in-time behind the
first A groups; x slices and ST tiles are double-buffered.
"""

import os

import numpy as np
import ml_dtypes

import concourse.bass as bass
import concourse.mybir as mybir
import concourse.tile as tile
from concourse import bacc
from concourse.bass_utils import run_bass_kernel_spmd

E, N, D, DFF = 8, 4096, 1024, 2048
P = 128
NT = N // P     # 32  token tiles
DC = D // P     # 8   d chunks (K for phase A)
FT = DFF // P   # 16  f chunks
DS = D // 512   # 2   d slices of 512
NS = N // 512   # 8   n slices of 512
SPT = 4         # token tiles per n slice

F32 = mybir.dt.float32
BF16 = mybir.dt.bfloat16
RELU = mybir.ActivationFunctionType.Relu
BF = ml_dtypes.bfloat16

_cache = {}


def _build():
    nc = bacc.Bacc()

    xT = nc.dram_tensor("xT", [D, N], BF16, kind="ExternalInput")
    # W1H[ft, p, dc, f'] = W1[dc*128 + p, ft*128 + f']  (host-tiled: one
    # contiguous chunk per f-tile so W1 can be staged just-in-time)
    W1H = nc.dram_tensor("W1H", [FT, P, DC, P], BF16, kind="ExternalInput")
    W2 = nc.dram_tensor("W2", [DFF, D], BF16, kind="ExternalInput")
    # ST[nt, p, sc, n'] = S_T[sc*128 + p, nt*128 + n']  (host-tiled so each
    # phase-C load is one contiguous 8KB line per partition)
    ST = nc.dram_tensor("ST", [NT, P, NT, P], BF16, kind="ExternalInput")
    out = nc.dram_tensor("out", [N, D], F32, kind="ExternalOutput")

    with tile.TileContext(nc) as tc:
        xTr = xT.rearrange("(dc p) n -> p dc n", p=P)
        W2r = W2.rearrange("(fc p) d -> p fc d", p=P)

        with tc.tile_pool(name="mp", bufs=1) as mp:
            msb = [None] * NT

            # ---------- fused phases A+B per 512-token slice ----------
            with tc.tile_pool(name="w1p", bufs=1) as w1p, \
                 tc.tile_pool(name="w2p", bufs=1) as w2p, \
                 tc.tile_pool(name="xp", bufs=2) as xp, \
                 tc.tile_pool(name="hp", bufs=2) as hp, \
                 tc.tile_pool(name="psA", bufs=3, space="PSUM") as psA, \
                 tc.tile_pool(name="psB", bufs=4, space="PSUM") as psB:
                w1sb = w1p.tile([P, FT, DC, P], BF16, name="w1sb")
                w2sb = w2p.tile([P, FT, D], BF16, name="w2sb")
                xsbs = [xp.tile([P, DC, 512], BF16, tag="xsb", name=f"xsb{i}") for i in range(NS)]
                nc.sync.dma_start(out=xsbs[0][:], in_=xTr[:, :, 0:512])
                nc.sync.dma_start(out=w1sb[:, 0], in_=W1H[0])
                for ns in range(NS):
                    # phase A: hT chunks for this slice (per-chunk tiles so
                    # phase B's deps are exact)
                    hs = [hp.tile([P, 512], BF16, tag=f"h{ft}", name=f"h{ft}_{ns}") for ft in range(FT)]
                    for ft in range(FT):
                        pt = psA.tile([P, 512], F32, tag="ptA", name="ptA")
                        for dc in range(DC):
                            nc.tensor.matmul(
                                out=pt[:],
                                lhsT=w1sb[:, ft, dc],
                                rhs=xsbs[ns][:, dc],
                                start=(dc == 0),
                                stop=(dc == DC - 1),
                            )
                        nc.scalar.activation(out=hs[ft][:], in_=pt[:], func=RELU)
                        if ns == 0:
                            # JIT-stage the rest of W1, then W2, behind the
                            # first slice's compute
                            if ft + 1 < FT:
                                nc.sync.dma_start(
                                    out=w1sb[:, ft + 1], in_=W1H[ft + 1]
                                )
                            nc.sync.dma_start(out=w2sb[:, ft], in_=W2r[:, ft])
                        if ft == 0 and ns + 1 < NS:
                            nc.sync.dma_start(
                                out=xsbs[ns + 1][:],
                                in_=xTr[:, :, (ns + 1) * 512 : (ns + 2) * 512],
                            )
                    # phase B: m tiles for this slice (ds pair shares the
                    # stationary h block per fc step)
                    for t in range(SPT):
                        nt = ns * SPT + t
                        msb[nt] = mp.tile([P, D], BF16, tag=f"m{nt}", name=f"m{nt}")
                        pts = [psB.tile([P, 512], F32, tag="ptB", name="ptB") for i in range(DS)]
                        for fc in range(FT):
                            for ds in range(DS):
                                nc.tensor.matmul(
                                    out=pts[ds][:],
                                    lhsT=hs[fc][:, t * P : (t + 1) * P],
                                    rhs=w2sb[:, fc, ds * 512 : (ds + 1) * 512],
                                    start=(fc == 0),
                                    stop=(fc == FT - 1),
                                )
                        for ds in range(DS):
                            nc.vector.tensor_copy(
                                out=msb[nt][:, ds * 512 : (ds + 1) * 512],
                                in_=pts[ds][:],
                            )

            # ---------- phase C: out = ST.T @ m ----------
            with tc.tile_pool(name="stp", bufs=2) as stp, \
                 tc.tile_pool(name="op", bufs=3) as op, \
                 tc.tile_pool(name="psC", bufs=4, space="PSUM") as psC:
                stsbs = [stp.tile([P, NT, P], BF16, tag="stsb", name=f"stsb{i}") for i in range(NT)]
                nc.sync.dma_start(out=stsbs[0][:], in_=ST[0])
                for nt in range(NT):
                    if nt + 1 < NT:
                        nc.sync.dma_start(out=stsbs[nt + 1][:], in_=ST[nt + 1])
                    pts = [psC.tile([P, 512], F32, tag="ptC", name="ptC") for i in range(DS)]
                    for sc in range(NT):
                        for ds in range(DS):
                            nc.tensor.matmul(
                                out=pts[ds][:],
                                lhsT=stsbs[nt][:, sc],
                                rhs=msb[sc][:, ds * 512 : (ds + 1) * 512],
                                start=(sc == 0),
                                stop=(sc == NT - 1),
                            )
                    for ds in range(DS):
                        osb = op.tile([P, 512], F32, tag="osb", name="osb")
                        nc.vector.tensor_copy(out=osb[:], in_=pts[ds][:])
                        nc.sync.dma_start(
                            out=out[nt * P : (nt + 1) * P, ds * 512 : (ds + 1) * 512],
                            in_=osb[:],
                        )

    nc.compile()
    return nc


def kernel(x, W1, W2, edge_index):
    x = np.asarray(x, dtype=np.float32)
    W1 = np.asarray(W1, dtype=np.float32)
    W2 = np.asarray(W2, dtype=np.float32)
    edge_index = np.asarray(edge_index)

    # S_T[s, d] = #edges with src==s and dst==d  (so out = S_T.T @ m)
    src = edge_index[0].astype(np.int64)
    dst = edge_index[1].astype(np.int64)
    counts = np.bincount(src * N + dst, minlength=N * N)
    S_T = counts.reshape(N, N).astype(np.float32)
    # host tiling for contiguous phase-C DMA: [nt, p, sc, n']
    STH = np.ascontiguousarray(
        S_T.reshape(NT, P, NT, P).transpose(2, 1, 0, 3)
    ).astype(BF)

    if "nc" not in _cache:
        _cache["nc"] = _build()
    nc = _cache["nc"]

    in_maps = []
    for e in range(E):
        # W1H[ft, p, dc, f'] = W1[e, dc*128+p, ft*128+f']
        W1H = np.ascontiguousarray(
            W1[e].reshape(DC, P, FT, P).transpose(2, 1, 0, 3)
        ).astype(BF)
        in_maps.append(
            {
                "xT": np.ascontiguousarray(x[e].T).astype(BF),
                "W1H": W1H,
                "W2": W2[e].astype(BF),
                "ST": STH,
            }
        )

    trace = bool(int(os.environ.get("PROBLEM_TRACE", "0")))
    res = run_bass_kernel_spmd(nc, in_maps, core_ids=list(range(E)), trace=trace)
    _cache["last_results"] = res
    return np.stack([res.results[e]["out"] for e in range(E)]).astype(np.float32)
